# revision 10
# baseline (speedup 1.0000x reference)
"""ConvSwiGLU Trainium2 kernel: tensor-parallel over d_ff across 8 NeuronCores.

v2 design (from v1 profile: DVE was 100% busy on 754ns scalar_tensor_tensor
conv taps, ACT 72%, PE only ~70% dense; 462us total vs a ~327us bf16 PE
roofline):

  - All matmuls bf16 (f32r gave no PE-rate advantage; bf16 halves DMA/SBUF).
  - h lives in per-sequence SBUF slabs [128, 2054]: 2 zero halo cols, 2048
    tokens, 2 zero halo + 2 pad. Conv reads cross chunk boundaries natively;
    no halo copies, no edge tiles, no host-side edge matmuls. Halo cols are
    memset once; matmul1 output (ACT Identity+bias copy) only ever writes the
    interior, so zeros persist across the slab's reuse by seq s and s+2.
  - The depthwise conv avoids scalar_tensor_tensor entirely (it has no fast
    DVE modes -> 1x). Instead: per-tap premultiply via tensor_scalar (4x for
    aligned bf16 SBUF, 2x_2p when the odd-tap read is 2-byte-shifted) into
    half-sequence tiles stored so that the add tree reads only EVEN offsets,
    then tensor_tensor adds at 2x. Conv bias for the up side rides the p2
    premult's second scalar slot; gate side via the Silu activation bias.
  - Work is spread so PE (~20.5us/chunk) is the bottleneck: DVE does aligned
    premults + add tree + swiglu combine, ACT does psum->sbuf copies (+bias),
    Silu, and down-psum copies, GpSimd (no PSUM port) does the odd-tap-1
    premult and one add per tile from SBUF.
  - Down matmul unchanged: psum[m,t] = sum_f Wd[f,m] hact[f,t]; per-core
    partial yT summed on the host (bf16 partials, f32 host accumulate).
"""

import os
import sys
from contextlib import ExitStack

import ml_dtypes
import numpy as np

for _p in ("/root/.axon_site/_ro/trn_rl_repo", "/opt/trn_rl_repo"):
    if os.path.isdir(_p) and _p not in sys.path:
        sys.path.append(_p)

import concourse.bass as bass
import concourse.tile as tile
from concourse import bacc, mybir
from concourse.bass_utils import run_bass_kernel_spmd

F32 = mybir.dt.float32
BF16 = mybir.dt.bfloat16
AF = mybir.ActivationFunctionType
ALU = mybir.AluOpType

B, L, D = 4, 2048, 1024
F = 4096
NCORES = 8
FS = F // NCORES          # 512 d_ff channels per core
KSUB = D // 128           # 8 contraction subtiles for gate/up
GRP = FS // 128           # 4 channel groups per core
MSUB = D // 128           # 8 output row subtiles for down matmul
T = 512                   # token chunk (psum bank limit for f32)
CPS = L // T              # 4 chunks per sequence
NCH = (B * L) // T        # 16 chunks
NSEQ = B                  # 4 sequences
K = 5                     # conv taps
SLAB_W = L + 6            # 2 halo + 2048 tokens + 2 halo + 2 pad
HMW = L // 2              # half-sequence width for conv ops (1024)
PW = HMW + 4              # premult tile width (1028)

_cache = {}


def _build_program():
    nc = bacc.Bacc("TRN2", target_bir_lowering=False, debug=False,
                   enable_asserts=False, num_devices=NCORES)

    xTc = nc.dram_tensor("xTc", [NCH, 128, KSUB, T], BF16, kind="ExternalInput").ap()
    wg = nc.dram_tensor("wgS", [128, KSUB, FS], BF16, kind="ExternalInput").ap()
    wu = nc.dram_tensor("wuS", [128, KSUB, FS], BF16, kind="ExternalInput").ap()
    wd = nc.dram_tensor("wdS", [128, GRP, D], BF16, kind="ExternalInput").ap()
    bg = nc.dram_tensor("bgS", [128, GRP], F32, kind="ExternalInput").ap()
    bu = nc.dram_tensor("buS", [128, GRP], F32, kind="ExternalInput").ap()
    cgw = nc.dram_tensor("cgwS", [128, GRP, K], F32, kind="ExternalInput").ap()
    cuw = nc.dram_tensor("cuwS", [128, GRP, K], F32, kind="ExternalInput").ap()
    cgb = nc.dram_tensor("cgbS", [128, GRP], F32, kind="ExternalInput").ap()
    cub = nc.dram_tensor("cubS", [128, GRP], F32, kind="ExternalInput").ap()
    yT = nc.dram_tensor("yT", [D, B * L], BF16, kind="ExternalOutput").ap()

    with tile.TileContext(nc) as tc, ExitStack() as ctx:
        consts = ctx.enter_context(tc.tile_pool(name="consts", bufs=1))
        xpool = ctx.enter_context(tc.tile_pool(name="x", bufs=3))
        ppool = ctx.enter_context(tc.tile_pool(name="p", bufs=2))
        abpool = ctx.enter_context(tc.tile_pool(name="ab", bufs=2))
        outpool = ctx.enter_context(tc.tile_pool(name="out", bufs=2))
        ps_main = ctx.enter_context(tc.tile_pool(name="psm", bufs=4, space="PSUM"))
        ps_dn = ctx.enter_context(tc.tile_pool(name="psd", bufs=4, space="PSUM"))

        # resident weights / constants
        wg_sb = consts.tile([128, KSUB, FS], BF16)
        wu_sb = consts.tile([128, KSUB, FS], BF16)
        wd_sb = consts.tile([128, GRP, D], BF16)
        bg_sb = consts.tile([128, GRP], F32)
        bu_sb = consts.tile([128, GRP], F32)
        cgw_sb = consts.tile([128, GRP, K], F32)
        cuw_sb = consts.tile([128, GRP, K], F32)
        cgb_sb = consts.tile([128, GRP], F32)
        cub_sb = consts.tile([128, GRP], F32)
        # const loads on the Activation DMA queue (x/out use the SP queue);
        # wg + smalls first so chunk 0 matmuls can start as early as possible
        nc.scalar.dma_start(wg_sb[:, 0:2], wg[:, 0:2])
        nc.scalar.dma_start(wg_sb[:, 2:4], wg[:, 2:4])
        nc.scalar.dma_start(wg_sb[:, 4:8], wg[:, 4:8])
        for sb, dr in ((bg_sb, bg), (bu_sb, bu), (cgw_sb, cgw), (cuw_sb, cuw),
                       (cgb_sb, cgb), (cub_sb, cub)):
            nc.scalar.dma_start(sb[:], dr)
        nc.scalar.dma_start(wu_sb[:], wu)
        nc.scalar.dma_start(wd_sb[:], wd)

        # persistent h slabs (2 sequence sets x 4 groups x 2 sides) and hact
        # slabs (2 sets x 4 groups); halo zeros written once
        hs = {}
        ha = {}
        for p in range(2):
            for g in range(GRP):
                for sd in range(2):
                    t_ = consts.tile([128, SLAB_W], BF16, name=f"hs{p}_{g}_{sd}", tag=f"hs{p}_{g}_{sd}")
                    nc.gpsimd.memset(t_[:, 0:2], 0.0)
                    nc.gpsimd.memset(t_[:, 2 + L:SLAB_W], 0.0)
                    hs[p, g, sd] = t_
                ha[p, g] = consts.tile([128, L], BF16, name=f"ha{p}_{g}", tag=f"ha{p}_{g}")

        def produce(i):
            """gate/up matmuls for chunk i -> biased h slab columns."""
            s, c = divmod(i, CPS)
            p = s % 2
            xt = xpool.tile([128, KSUB, T], BF16, tag="xt")
            nc.sync.dma_start(xt[:], xTc[i])
            for g in range(GRP):
                for sd, (w_sb, b_sb) in enumerate(((wg_sb, bg_sb),
                                                   (wu_sb, bu_sb))):
                    ps = ps_main.tile([128, T], F32, tag="mm1")
                    for ks in range(KSUB):
                        nc.tensor.matmul(ps[:],
                                         w_sb[:, ks, g * 128:(g + 1) * 128],
                                         xt[:, ks, :],
                                         start=(ks == 0), stop=(ks == KSUB - 1))
                    nc.scalar.activation(
                        hs[p, g, sd][:, 2 + c * T:2 + (c + 1) * T], ps[:],
                        AF.Identity, bias=b_sb[:, g:g + 1])

        def conv_range(s, t0, W):
            """conv + swiglu for tokens [t0, t0+W) of sequence s."""
            p = s % 2
            for g in range(GRP):
                rs = []
                for sd, tw in enumerate((cgw_sb, cuw_sb)):
                    slab = hs[p, g, sd]
                    P = []
                    for j in range(K):
                        dlt = j % 2  # odd taps read 1 col shifted (2x_2p)
                        src = slab[:, t0 + dlt:t0 + dlt + W + 4]
                        pt = ppool.tile([128, PW], BF16, name=f"p{j}", tag=f"p{j}")[:, 0:W + 4]
                        w_ap = tw[:, g, j:j + 1]
                        if j == 2:
                            # center tap on ACT (per-partition scale is free
                            # there); fold conv-u bias via the bias slot --
                            # p2's read window never touches the halo so the
                            # +cub is interior-only
                            bias = cub_sb[:, g:g + 1] if sd == 1 else 0.0
                            nc.scalar.activation(pt[:], src, AF.Identity,
                                                 bias=bias, scale=w_ap)
                        else:
                            nc.vector.tensor_scalar(pt[:], src, w_ap, None,
                                                    ALU.mult)
                        P.append(pt)
                    a = abpool.tile([128, HMW], BF16, name="ta", tag="ta")[:, 0:W]
                    nc.vector.tensor_tensor(a[:], P[0][:, 0:W],
                                            P[4][:, 4:4 + W], ALU.add)
                    b = abpool.tile([128, HMW], BF16, name="tb", tag="tb")[:, 0:W]
                    nc.gpsimd.tensor_tensor(b[:], P[1][:, 0:W],
                                            P[3][:, 2:2 + W], ALU.add)
                    cc = abpool.tile([128, HMW], BF16, name="tc", tag="tc")[:, 0:W]
                    nc.vector.tensor_tensor(cc[:], a[:], b[:], ALU.add)
                    r = abpool.tile([128, HMW], BF16, name=f"tr{sd}", tag=f"tr{sd}")[:, 0:W]
                    nc.vector.tensor_tensor(r[:], cc[:], P[2][:, 2:2 + W],
                                            ALU.add)
                    rs.append(r)
                gact = abpool.tile([128, HMW], BF16, name="tga", tag="tga")[:, 0:W]
                nc.scalar.activation(gact[:], rs[0][:], AF.Silu,
                                     bias=cgb_sb[:, g:g + 1])
                nc.vector.tensor_tensor(ha[p, g][:, t0:t0 + W], gact[:],
                                        rs[1][:], ALU.mult)

        def down(i):
            """down matmul + output DMA for chunk i."""
            s, c = divmod(i, CPS)
            p = s % 2
            out_sb = outpool.tile([128, MSUB, T], BF16, tag="out")
            for ms in range(MSUB):
                dps = ps_dn.tile([128, T], F32, tag="dn")
                for g in range(GRP):
                    nc.tensor.matmul(dps[:],
                                     wd_sb[:, g, ms * 128:(ms + 1) * 128],
                                     ha[p, g][:, c * T:(c + 1) * T],
                                     start=(g == 0), stop=(g == GRP - 1))
                nc.scalar.copy(out_sb[:, ms, :], dps[:])
            nc.gpsimd.dma_start(
                yT.rearrange("(ms p) t -> p ms t", p=128)[:, :, i * T:(i + 1) * T],
                out_sb[:])

        for s in range(NSEQ - 1):
            for c in range(CPS):
                produce(CPS * s + c)
                if c == 2:
                    # half 0 conv needs slab cols up to 1028 (first cols of
                    # chunk 2) -> emit after chunk 2's copies
                    conv_range(s, 0, HMW)
                    down(CPS * s + 0)
                    down(CPS * s + 1)
            conv_range(s, HMW, HMW)
            down(CPS * s + 2)
            down(CPS * s + 3)
        # last sequence at chunk granularity to shorten the drain tail:
        # conv of chunk c needs the first cols of chunk c+1's copies
        s = NSEQ - 1
        produce(CPS * s + 0)
        produce(CPS * s + 1)
        conv_range(s, 0, T)
        produce(CPS * s + 2)
        conv_range(s, T, T)
        down(CPS * s + 0)
        produce(CPS * s + 3)
        conv_range(s, 2 * T, T)
        down(CPS * s + 1)
        conv_range(s, 3 * T, T)
        down(CPS * s + 2)
        down(CPS * s + 3)

    nc.compile()
    return nc


def _prep_inputs(x, Wg, bgv, Wu, buv, convg_w, convg_b, convu_w, convu_b, Wd):
    """Host-side shard/layout. Returns list of per-core in_maps."""
    bf16 = ml_dtypes.bfloat16
    x = np.ascontiguousarray(x, np.float32)
    # [B, L, D] -> [B, KSUB, 128, L] -> chunks [NCH, 128, KSUB, T]
    xt = x.transpose(0, 2, 1).reshape(B, KSUB, 128, L)
    xTc = np.stack([
        xt[i // CPS, :, :, (i % CPS) * T:(i % CPS + 1) * T].transpose(1, 0, 2)
        for i in range(NCH)
    ]).astype(bf16)

    def colsplit(w, c):      # [D, F] -> per-core [128, KSUB, FS]
        s = np.asarray(w, np.float32)[:, c * FS:(c + 1) * FS]
        return np.ascontiguousarray(
            s.reshape(KSUB, 128, FS).transpose(1, 0, 2)).astype(bf16)

    def vecsplit(v, c):      # [F] -> [128, GRP] f32
        return np.ascontiguousarray(
            np.asarray(v, np.float32)[c * FS:(c + 1) * FS].reshape(GRP, 128).T)

    def tapsplit(w, c):      # [F, 1, K] -> [128, GRP, K] f32
        return np.ascontiguousarray(
            np.asarray(w, np.float32)[c * FS:(c + 1) * FS, 0, :]
            .reshape(GRP, 128, K).transpose(1, 0, 2))

    in_maps = []
    for c in range(NCORES):
        wdS = np.asarray(Wd, np.float32)[c * FS:(c + 1) * FS, :]
        in_maps.append({
            "xTc": xTc,
            "wgS": colsplit(Wg, c),
            "wuS": colsplit(Wu, c),
            "wdS": np.ascontiguousarray(
                wdS.reshape(GRP, 128, D).transpose(1, 0, 2)).astype(bf16),
            "bgS": vecsplit(bgv, c),
            "buS": vecsplit(buv, c),
            "cgwS": tapsplit(convg_w, c),
            "cuwS": tapsplit(convu_w, c),
            "cgbS": vecsplit(convg_b, c),
            "cubS": vecsplit(convu_b, c),
        })
    return in_maps


def run_on_cores(in_maps, **kwargs):
    if "nc" not in _cache:
        _cache["nc"] = _build_program()
    return run_bass_kernel_spmd(_cache["nc"], in_maps,
                                core_ids=list(range(NCORES)), **kwargs)


def kernel(x, Wg, bg, Wu, bu, convg_w, convg_b, convu_w, convu_b, Wd, bd):
    in_maps = _prep_inputs(x, Wg, bg, Wu, bu, convg_w, convg_b,
                           convu_w, convu_b, Wd)
    res = run_on_cores(in_maps)
    acc = np.zeros((D, B * L), np.float32)
    for r in res.results:
        acc += np.asarray(r["yT"], np.float32)
    acc += np.asarray(bd, np.float32)[:, None]
    return np.ascontiguousarray(acc.T.reshape(B, L, D)).astype(np.float32)


# revision 11
# speedup vs baseline: 1.2534x; 1.2534x over previous
"""ConvSwiGLU Trainium2 kernel: tensor-parallel over d_ff across 8 NeuronCores.

v2 design (from v1 profile: DVE was 100% busy on 754ns scalar_tensor_tensor
conv taps, ACT 72%, PE only ~70% dense; 462us total vs a ~327us bf16 PE
roofline):

  - All matmuls bf16 (f32r gave no PE-rate advantage; bf16 halves DMA/SBUF).
  - h lives in per-sequence SBUF slabs [128, 2054]: 2 zero halo cols, 2048
    tokens, 2 zero halo + 2 pad. Conv reads cross chunk boundaries natively;
    no halo copies, no edge tiles, no host-side edge matmuls. Halo cols are
    memset once; matmul1 output (ACT Identity+bias copy) only ever writes the
    interior, so zeros persist across the slab's reuse by seq s and s+2.
  - The depthwise conv avoids scalar_tensor_tensor entirely (it has no fast
    DVE modes -> 1x). Instead: per-tap premultiply via tensor_scalar (4x for
    aligned bf16 SBUF, 2x_2p when the odd-tap read is 2-byte-shifted) into
    half-sequence tiles stored so that the add tree reads only EVEN offsets,
    then tensor_tensor adds at 2x. Conv bias for the up side rides the p2
    premult's second scalar slot; gate side via the Silu activation bias.
  - Work is spread so PE (~20.5us/chunk) is the bottleneck: DVE does aligned
    premults + add tree + swiglu combine, ACT does psum->sbuf copies (+bias),
    Silu, and down-psum copies, GpSimd (no PSUM port, and its software
    tensor ops are too slow for the critical path) only issues output DMAs.
  - Down matmul unchanged: psum[m,t] = sum_f Wd[f,m] hact[f,t]; per-core
    partial yT summed on the host (bf16 partials, f32 host accumulate).
"""

import os
import sys
from contextlib import ExitStack

import ml_dtypes
import numpy as np

for _p in ("/root/.axon_site/_ro/trn_rl_repo", "/opt/trn_rl_repo"):
    if os.path.isdir(_p) and _p not in sys.path:
        sys.path.append(_p)

import concourse.bass as bass
import concourse.tile as tile
from concourse import bacc, mybir
from concourse.bass_utils import run_bass_kernel_spmd

F32 = mybir.dt.float32
BF16 = mybir.dt.bfloat16
AF = mybir.ActivationFunctionType
ALU = mybir.AluOpType

B, L, D = 4, 2048, 1024
F = 4096
NCORES = 8
FS = F // NCORES          # 512 d_ff channels per core
KSUB = D // 128           # 8 contraction subtiles for gate/up
GRP = FS // 128           # 4 channel groups per core
MSUB = D // 128           # 8 output row subtiles for down matmul
T = 512                   # token chunk (psum bank limit for f32)
CPS = L // T              # 4 chunks per sequence
NCH = (B * L) // T        # 16 chunks
NSEQ = B                  # 4 sequences
K = 5                     # conv taps
SLAB_W = L + 6            # 2 halo + 2048 tokens + 2 halo + 2 pad
HMW = L // 2              # half-sequence width for conv ops (1024)
PW = HMW + 4              # premult tile width (1028)

_cache = {}


def _build_program():
    nc = bacc.Bacc("TRN2", target_bir_lowering=False, debug=False,
                   enable_asserts=False, num_devices=NCORES)

    xTc = nc.dram_tensor("xTc", [NCH, 128, KSUB, T], BF16, kind="ExternalInput").ap()
    wg = nc.dram_tensor("wgS", [128, KSUB, FS], BF16, kind="ExternalInput").ap()
    wu = nc.dram_tensor("wuS", [128, KSUB, FS], BF16, kind="ExternalInput").ap()
    wd = nc.dram_tensor("wdS", [128, GRP, D], BF16, kind="ExternalInput").ap()
    bg = nc.dram_tensor("bgS", [128, GRP], F32, kind="ExternalInput").ap()
    bu = nc.dram_tensor("buS", [128, GRP], F32, kind="ExternalInput").ap()
    cgw = nc.dram_tensor("cgwS", [128, GRP, K], F32, kind="ExternalInput").ap()
    cuw = nc.dram_tensor("cuwS", [128, GRP, K], F32, kind="ExternalInput").ap()
    cgb = nc.dram_tensor("cgbS", [128, GRP], F32, kind="ExternalInput").ap()
    cub = nc.dram_tensor("cubS", [128, GRP], F32, kind="ExternalInput").ap()
    yT = nc.dram_tensor("yT", [D, B * L], BF16, kind="ExternalOutput").ap()

    with tile.TileContext(nc) as tc, ExitStack() as ctx:
        consts = ctx.enter_context(tc.tile_pool(name="consts", bufs=1))
        xpool = ctx.enter_context(tc.tile_pool(name="x", bufs=3))
        ppool = ctx.enter_context(tc.tile_pool(name="p", bufs=2))
        abpool = ctx.enter_context(tc.tile_pool(name="ab", bufs=2))
        outpool = ctx.enter_context(tc.tile_pool(name="out", bufs=2))
        ps_main = ctx.enter_context(tc.tile_pool(name="psm", bufs=4, space="PSUM"))
        ps_dn = ctx.enter_context(tc.tile_pool(name="psd", bufs=4, space="PSUM"))

        # resident weights / constants
        wg_sb = consts.tile([128, KSUB, FS], BF16)
        wu_sb = consts.tile([128, KSUB, FS], BF16)
        wd_sb = consts.tile([128, GRP, D], BF16)
        bg_sb = consts.tile([128, GRP], F32)
        bu_sb = consts.tile([128, GRP], F32)
        cgw_sb = consts.tile([128, GRP, K], F32)
        cuw_sb = consts.tile([128, GRP, K], F32)
        cgb_sb = consts.tile([128, GRP], F32)
        cub_sb = consts.tile([128, GRP], F32)
        # const loads on the Activation DMA queue (x/out use the SP queue);
        # wg + smalls first so chunk 0 matmuls can start as early as possible
        nc.scalar.dma_start(wg_sb[:, 0:2], wg[:, 0:2])
        nc.scalar.dma_start(wg_sb[:, 2:4], wg[:, 2:4])
        nc.scalar.dma_start(wg_sb[:, 4:8], wg[:, 4:8])
        for sb, dr in ((bg_sb, bg), (bu_sb, bu), (cgw_sb, cgw), (cuw_sb, cuw),
                       (cgb_sb, cgb), (cub_sb, cub)):
            nc.scalar.dma_start(sb[:], dr)
        nc.scalar.dma_start(wu_sb[:], wu)
        nc.scalar.dma_start(wd_sb[:], wd)

        # persistent h slabs (2 sequence sets x 4 groups x 2 sides) and hact
        # slabs (2 sets x 4 groups); halo zeros written once
        hs = {}
        ha = {}
        for p in range(2):
            for g in range(GRP):
                for sd in range(2):
                    t_ = consts.tile([128, SLAB_W], BF16, name=f"hs{p}_{g}_{sd}", tag=f"hs{p}_{g}_{sd}")
                    nc.gpsimd.memset(t_[:, 0:2], 0.0)
                    nc.gpsimd.memset(t_[:, 2 + L:SLAB_W], 0.0)
                    hs[p, g, sd] = t_
                ha[p, g] = consts.tile([128, L], BF16, name=f"ha{p}_{g}", tag=f"ha{p}_{g}")

        def produce(i):
            """gate/up matmuls for chunk i -> biased h slab columns."""
            s, c = divmod(i, CPS)
            p = s % 2
            xt = xpool.tile([128, KSUB, T], BF16, tag="xt")
            nc.sync.dma_start(xt[:], xTc[i])
            for g in range(GRP):
                for sd, (w_sb, b_sb) in enumerate(((wg_sb, bg_sb),
                                                   (wu_sb, bu_sb))):
                    ps = ps_main.tile([128, T], F32, tag="mm1")
                    for ks in range(KSUB):
                        nc.tensor.matmul(ps[:],
                                         w_sb[:, ks, g * 128:(g + 1) * 128],
                                         xt[:, ks, :],
                                         start=(ks == 0), stop=(ks == KSUB - 1))
                    nc.scalar.activation(
                        hs[p, g, sd][:, 2 + c * T:2 + (c + 1) * T], ps[:],
                        AF.Identity, bias=b_sb[:, g:g + 1])

        def conv_range(s, t0, W):
            """conv + swiglu for tokens [t0, t0+W) of sequence s."""
            p = s % 2
            for g in range(GRP):
                rs = []
                for sd, tw in enumerate((cgw_sb, cuw_sb)):
                    slab = hs[p, g, sd]
                    P = []
                    for j in range(K):
                        dlt = j % 2  # odd taps read 1 col shifted (2x_2p)
                        src = slab[:, t0 + dlt:t0 + dlt + W + 4]
                        pt = ppool.tile([128, PW], BF16, name=f"p{j}", tag=f"p{j}")[:, 0:W + 4]
                        w_ap = tw[:, g, j:j + 1]
                        if j == 2:
                            # center tap on ACT (per-partition scale is free
                            # there); fold conv-u bias via the bias slot --
                            # p2's read window never touches the halo so the
                            # +cub is interior-only
                            bias = cub_sb[:, g:g + 1] if sd == 1 else 0.0
                            nc.scalar.activation(pt[:], src, AF.Identity,
                                                 bias=bias, scale=w_ap)
                        else:
                            nc.vector.tensor_scalar(pt[:], src, w_ap, None,
                                                    ALU.mult)
                        P.append(pt)
                    a = abpool.tile([128, HMW], BF16, name="ta", tag="ta")[:, 0:W]
                    nc.vector.tensor_tensor(a[:], P[0][:, 0:W],
                                            P[4][:, 4:4 + W], ALU.add)
                    b = abpool.tile([128, HMW], BF16, name="tb", tag="tb")[:, 0:W]
                    nc.vector.tensor_tensor(b[:], P[1][:, 0:W],
                                            P[3][:, 2:2 + W], ALU.add)
                    cc = abpool.tile([128, HMW], BF16, name="tc", tag="tc")[:, 0:W]
                    nc.vector.tensor_tensor(cc[:], a[:], b[:], ALU.add)
                    r = abpool.tile([128, HMW], BF16, name=f"tr{sd}", tag=f"tr{sd}")[:, 0:W]
                    nc.vector.tensor_tensor(r[:], cc[:], P[2][:, 2:2 + W],
                                            ALU.add)
                    rs.append(r)
                gact = abpool.tile([128, HMW], BF16, name="tga", tag="tga")[:, 0:W]
                nc.scalar.activation(gact[:], rs[0][:], AF.Silu,
                                     bias=cgb_sb[:, g:g + 1])
                nc.vector.tensor_tensor(ha[p, g][:, t0:t0 + W], gact[:],
                                        rs[1][:], ALU.mult)

        def down(i):
            """down matmul + output DMA for chunk i."""
            s, c = divmod(i, CPS)
            p = s % 2
            out_sb = outpool.tile([128, MSUB, T], BF16, tag="out")
            for ms in range(MSUB):
                dps = ps_dn.tile([128, T], F32, tag="dn")
                for g in range(GRP):
                    nc.tensor.matmul(dps[:],
                                     wd_sb[:, g, ms * 128:(ms + 1) * 128],
                                     ha[p, g][:, c * T:(c + 1) * T],
                                     start=(g == 0), stop=(g == GRP - 1))
                nc.scalar.copy(out_sb[:, ms, :], dps[:])
            nc.gpsimd.dma_start(
                yT.rearrange("(ms p) t -> p ms t", p=128)[:, :, i * T:(i + 1) * T],
                out_sb[:])

        for s in range(NSEQ - 1):
            for c in range(CPS):
                produce(CPS * s + c)
                if c == 2:
                    # half 0 conv needs slab cols up to 1028 (first cols of
                    # chunk 2) -> emit after chunk 2's copies
                    conv_range(s, 0, HMW)
                    down(CPS * s + 0)
                    down(CPS * s + 1)
            conv_range(s, HMW, HMW)
            down(CPS * s + 2)
            down(CPS * s + 3)
        # last sequence at chunk granularity to shorten the drain tail:
        # conv of chunk c needs the first cols of chunk c+1's copies
        s = NSEQ - 1
        produce(CPS * s + 0)
        produce(CPS * s + 1)
        conv_range(s, 0, T)
        produce(CPS * s + 2)
        conv_range(s, T, T)
        down(CPS * s + 0)
        produce(CPS * s + 3)
        conv_range(s, 2 * T, T)
        down(CPS * s + 1)
        conv_range(s, 3 * T, T)
        down(CPS * s + 2)
        down(CPS * s + 3)

    nc.compile()
    return nc


def _prep_inputs(x, Wg, bgv, Wu, buv, convg_w, convg_b, convu_w, convu_b, Wd):
    """Host-side shard/layout. Returns list of per-core in_maps."""
    bf16 = ml_dtypes.bfloat16
    x = np.ascontiguousarray(x, np.float32)
    # [B, L, D] -> [B, KSUB, 128, L] -> chunks [NCH, 128, KSUB, T]
    xt = x.transpose(0, 2, 1).reshape(B, KSUB, 128, L)
    xTc = np.stack([
        xt[i // CPS, :, :, (i % CPS) * T:(i % CPS + 1) * T].transpose(1, 0, 2)
        for i in range(NCH)
    ]).astype(bf16)

    def colsplit(w, c):      # [D, F] -> per-core [128, KSUB, FS]
        s = np.asarray(w, np.float32)[:, c * FS:(c + 1) * FS]
        return np.ascontiguousarray(
            s.reshape(KSUB, 128, FS).transpose(1, 0, 2)).astype(bf16)

    def vecsplit(v, c):      # [F] -> [128, GRP] f32
        return np.ascontiguousarray(
            np.asarray(v, np.float32)[c * FS:(c + 1) * FS].reshape(GRP, 128).T)

    def tapsplit(w, c):      # [F, 1, K] -> [128, GRP, K] f32
        return np.ascontiguousarray(
            np.asarray(w, np.float32)[c * FS:(c + 1) * FS, 0, :]
            .reshape(GRP, 128, K).transpose(1, 0, 2))

    in_maps = []
    for c in range(NCORES):
        wdS = np.asarray(Wd, np.float32)[c * FS:(c + 1) * FS, :]
        in_maps.append({
            "xTc": xTc,
            "wgS": colsplit(Wg, c),
            "wuS": colsplit(Wu, c),
            "wdS": np.ascontiguousarray(
                wdS.reshape(GRP, 128, D).transpose(1, 0, 2)).astype(bf16),
            "bgS": vecsplit(bgv, c),
            "buS": vecsplit(buv, c),
            "cgwS": tapsplit(convg_w, c),
            "cuwS": tapsplit(convu_w, c),
            "cgbS": vecsplit(convg_b, c),
            "cubS": vecsplit(convu_b, c),
        })
    return in_maps


def run_on_cores(in_maps, **kwargs):
    if "nc" not in _cache:
        _cache["nc"] = _build_program()
    return run_bass_kernel_spmd(_cache["nc"], in_maps,
                                core_ids=list(range(NCORES)), **kwargs)


def kernel(x, Wg, bg, Wu, bu, convg_w, convg_b, convu_w, convu_b, Wd, bd):
    in_maps = _prep_inputs(x, Wg, bg, Wu, bu, convg_w, convg_b,
                           convu_w, convu_b, Wd)
    res = run_on_cores(in_maps)
    acc = np.zeros((D, B * L), np.float32)
    for r in res.results:
        acc += np.asarray(r["yT"], np.float32)
    acc += np.asarray(bd, np.float32)[:, None]
    return np.ascontiguousarray(acc.T.reshape(B, L, D)).astype(np.float32)


# revision 12
# speedup vs baseline: 1.2555x; 1.0016x over previous
"""ConvSwiGLU Trainium2 kernel: tensor-parallel over d_ff across 8 NeuronCores.

v2 design (from v1 profile: DVE was 100% busy on 754ns scalar_tensor_tensor
conv taps, ACT 72%, PE only ~70% dense; 462us total vs a ~327us bf16 PE
roofline):

  - All matmuls bf16 (f32r gave no PE-rate advantage; bf16 halves DMA/SBUF).
  - h lives in per-sequence SBUF slabs [128, 2054]: 2 zero halo cols, 2048
    tokens, 2 zero halo + 2 pad. Conv reads cross chunk boundaries natively;
    no halo copies, no edge tiles, no host-side edge matmuls. Halo cols are
    memset once; matmul1 output (ACT Identity+bias copy) only ever writes the
    interior, so zeros persist across the slab's reuse by seq s and s+2.
  - The depthwise conv avoids scalar_tensor_tensor entirely (it has no fast
    DVE modes -> 1x). Instead: per-tap premultiply via tensor_scalar (4x for
    aligned bf16 SBUF, 2x_2p when the odd-tap read is 2-byte-shifted) into
    half-sequence tiles stored so that the add tree reads only EVEN offsets,
    then tensor_tensor adds at 2x. Conv bias for the up side rides the p2
    premult's second scalar slot; gate side via the Silu activation bias.
  - Work is spread so PE (~20.5us/chunk) is the bottleneck: DVE does aligned
    premults + add tree + swiglu combine, ACT does psum->sbuf copies (+bias),
    Silu, and down-psum copies, GpSimd (no PSUM port, and its software
    tensor ops are too slow for the critical path) only issues output DMAs.
  - Down matmul unchanged: psum[m,t] = sum_f Wd[f,m] hact[f,t]; per-core
    partial yT summed on the host (bf16 partials, f32 host accumulate).
"""

import os
import sys
from contextlib import ExitStack

import ml_dtypes
import numpy as np

for _p in ("/root/.axon_site/_ro/trn_rl_repo", "/opt/trn_rl_repo"):
    if os.path.isdir(_p) and _p not in sys.path:
        sys.path.append(_p)

import concourse.bass as bass
import concourse.tile as tile
from concourse import bacc, mybir
from concourse.bass_utils import run_bass_kernel_spmd

F32 = mybir.dt.float32
BF16 = mybir.dt.bfloat16
AF = mybir.ActivationFunctionType
ALU = mybir.AluOpType

B, L, D = 4, 2048, 1024
F = 4096
NCORES = 8
FS = F // NCORES          # 512 d_ff channels per core
KSUB = D // 128           # 8 contraction subtiles for gate/up
GRP = FS // 128           # 4 channel groups per core
MSUB = D // 128           # 8 output row subtiles for down matmul
T = 512                   # token chunk (psum bank limit for f32)
CPS = L // T              # 4 chunks per sequence
NCH = (B * L) // T        # 16 chunks
NSEQ = B                  # 4 sequences
K = 5                     # conv taps
SLAB_W = L + 6            # 2 halo + 2048 tokens + 2 halo + 2 pad
HMW = L // 2              # half-sequence width for conv ops (1024)
PW = HMW + 4              # premult tile width (1028)

_cache = {}


def _build_program():
    nc = bacc.Bacc("TRN2", target_bir_lowering=False, debug=False,
                   enable_asserts=False, num_devices=NCORES)

    xTc = nc.dram_tensor("xTc", [NCH, 128, KSUB, T], BF16, kind="ExternalInput").ap()
    wg = nc.dram_tensor("wgS", [128, KSUB, FS], BF16, kind="ExternalInput").ap()
    wu = nc.dram_tensor("wuS", [128, KSUB, FS], BF16, kind="ExternalInput").ap()
    wd = nc.dram_tensor("wdS", [128, GRP, D], BF16, kind="ExternalInput").ap()
    bg = nc.dram_tensor("bgS", [128, GRP], F32, kind="ExternalInput").ap()
    bu = nc.dram_tensor("buS", [128, GRP], F32, kind="ExternalInput").ap()
    cgw = nc.dram_tensor("cgwS", [128, GRP, K], F32, kind="ExternalInput").ap()
    cuw = nc.dram_tensor("cuwS", [128, GRP, K], F32, kind="ExternalInput").ap()
    cgb = nc.dram_tensor("cgbS", [128, GRP], F32, kind="ExternalInput").ap()
    cub = nc.dram_tensor("cubS", [128, GRP], F32, kind="ExternalInput").ap()
    yT = nc.dram_tensor("yT", [D, B * L], BF16, kind="ExternalOutput").ap()

    with tile.TileContext(nc) as tc, ExitStack() as ctx:
        consts = ctx.enter_context(tc.tile_pool(name="consts", bufs=1))
        xpool = ctx.enter_context(tc.tile_pool(name="x", bufs=3))
        ppool = ctx.enter_context(tc.tile_pool(name="p", bufs=2))
        abpool = ctx.enter_context(tc.tile_pool(name="ab", bufs=2))
        outpool = ctx.enter_context(tc.tile_pool(name="out", bufs=2))
        ps_main = ctx.enter_context(tc.tile_pool(name="psm", bufs=4, space="PSUM"))
        ps_dn = ctx.enter_context(tc.tile_pool(name="psd", bufs=4, space="PSUM"))

        # resident weights / constants
        wg_sb = consts.tile([128, KSUB, FS], BF16)
        wu_sb = consts.tile([128, KSUB, FS], BF16)
        wd_sb = consts.tile([128, GRP, D], BF16)
        bg_sb = consts.tile([128, GRP], F32)
        bu_sb = consts.tile([128, GRP], F32)
        cgw_sb = consts.tile([128, GRP, K], F32)
        cuw_sb = consts.tile([128, GRP, K], F32)
        cgb_sb = consts.tile([128, GRP], F32)
        cub_sb = consts.tile([128, GRP], F32)
        # const loads on the Activation DMA queue (x/out use the SP queue);
        # wg + smalls first so chunk 0 matmuls can start as early as possible
        nc.scalar.dma_start(wg_sb[:, 0:2], wg[:, 0:2])
        nc.scalar.dma_start(wg_sb[:, 2:4], wg[:, 2:4])
        nc.scalar.dma_start(wg_sb[:, 4:8], wg[:, 4:8])
        for sb, dr in ((bg_sb, bg), (bu_sb, bu), (cgw_sb, cgw), (cuw_sb, cuw),
                       (cgb_sb, cgb), (cub_sb, cub)):
            nc.scalar.dma_start(sb[:], dr)
        nc.scalar.dma_start(wu_sb[:], wu)
        nc.scalar.dma_start(wd_sb[:], wd)

        # persistent h slabs (2 sequence sets x 4 groups x 2 sides) and hact
        # slabs (2 sets x 4 groups); halo zeros written once
        hs = {}
        ha = {}
        for p in range(2):
            for g in range(GRP):
                for sd in range(2):
                    t_ = consts.tile([128, SLAB_W], BF16, name=f"hs{p}_{g}_{sd}", tag=f"hs{p}_{g}_{sd}")
                    nc.gpsimd.memset(t_[:, 0:2], 0.0)
                    nc.gpsimd.memset(t_[:, 2 + L:SLAB_W], 0.0)
                    hs[p, g, sd] = t_
                ha[p, g] = consts.tile([128, L], BF16, name=f"ha{p}_{g}", tag=f"ha{p}_{g}")

        def produce(i):
            """gate/up matmuls for chunk i -> biased h slab columns."""
            s, c = divmod(i, CPS)
            p = s % 2
            xt = xpool.tile([128, KSUB, T], BF16, tag="xt")
            nc.sync.dma_start(xt[:], xTc[i])
            for g in range(GRP):
                for sd, (w_sb, b_sb) in enumerate(((wg_sb, bg_sb),
                                                   (wu_sb, bu_sb))):
                    ps = ps_main.tile([128, T], F32, tag="mm1")
                    for ks in range(KSUB):
                        nc.tensor.matmul(ps[:],
                                         w_sb[:, ks, g * 128:(g + 1) * 128],
                                         xt[:, ks, :],
                                         start=(ks == 0), stop=(ks == KSUB - 1))
                    nc.scalar.activation(
                        hs[p, g, sd][:, 2 + c * T:2 + (c + 1) * T], ps[:],
                        AF.Identity, bias=b_sb[:, g:g + 1])

        def conv_range(s, t0, W):
            """conv + swiglu for tokens [t0, t0+W) of sequence s."""
            p = s % 2
            for g in range(GRP):
                rs = []
                for sd, tw in enumerate((cgw_sb, cuw_sb)):
                    slab = hs[p, g, sd]
                    P = []
                    for j in range(K):
                        dlt = j % 2  # odd taps read 1 col shifted (2x_2p)
                        src = slab[:, t0 + dlt:t0 + dlt + W + 4]
                        pt = ppool.tile([128, PW], BF16, name=f"p{j}", tag=f"p{j}")[:, 0:W + 4]
                        w_ap = tw[:, g, j:j + 1]
                        if j == 2 or (j == 4 and sd == 0):
                            # center tap (+ gate-side tap 4) on ACT, which has
                            # slack and applies per-partition scale for free;
                            # fold conv-u bias via the bias slot -- p2's read
                            # window never touches the halo so +cub is
                            # interior-only
                            bias = cub_sb[:, g:g + 1] if j == 2 and sd == 1 else 0.0
                            nc.scalar.activation(pt[:], src, AF.Identity,
                                                 bias=bias, scale=w_ap)
                        else:
                            nc.vector.tensor_scalar(pt[:], src, w_ap, None,
                                                    ALU.mult)
                        P.append(pt)
                    a = abpool.tile([128, HMW], BF16, name="ta", tag="ta")[:, 0:W]
                    nc.vector.tensor_tensor(a[:], P[0][:, 0:W],
                                            P[4][:, 4:4 + W], ALU.add)
                    b = abpool.tile([128, HMW], BF16, name="tb", tag="tb")[:, 0:W]
                    nc.vector.tensor_tensor(b[:], P[1][:, 0:W],
                                            P[3][:, 2:2 + W], ALU.add)
                    cc = abpool.tile([128, HMW], BF16, name="tc", tag="tc")[:, 0:W]
                    nc.vector.tensor_tensor(cc[:], a[:], b[:], ALU.add)
                    r = abpool.tile([128, HMW], BF16, name=f"tr{sd}", tag=f"tr{sd}")[:, 0:W]
                    nc.vector.tensor_tensor(r[:], cc[:], P[2][:, 2:2 + W],
                                            ALU.add)
                    rs.append(r)
                gact = abpool.tile([128, HMW], BF16, name="tga", tag="tga")[:, 0:W]
                nc.scalar.activation(gact[:], rs[0][:], AF.Silu,
                                     bias=cgb_sb[:, g:g + 1])
                nc.vector.tensor_tensor(ha[p, g][:, t0:t0 + W], gact[:],
                                        rs[1][:], ALU.mult)

        def down(i):
            """down matmul + output DMA for chunk i."""
            s, c = divmod(i, CPS)
            p = s % 2
            out_sb = outpool.tile([128, MSUB, T], BF16, tag="out")
            for ms in range(MSUB):
                dps = ps_dn.tile([128, T], F32, tag="dn")
                for g in range(GRP):
                    nc.tensor.matmul(dps[:],
                                     wd_sb[:, g, ms * 128:(ms + 1) * 128],
                                     ha[p, g][:, c * T:(c + 1) * T],
                                     start=(g == 0), stop=(g == GRP - 1))
                nc.scalar.copy(out_sb[:, ms, :], dps[:])
            nc.gpsimd.dma_start(
                yT.rearrange("(ms p) t -> p ms t", p=128)[:, :, i * T:(i + 1) * T],
                out_sb[:])

        for s in range(NSEQ - 1):
            for c in range(CPS):
                produce(CPS * s + c)
                if c == 2:
                    # half 0 conv needs slab cols up to 1028 (first cols of
                    # chunk 2) -> emit after chunk 2's copies
                    conv_range(s, 0, HMW)
                    down(CPS * s + 0)
                    down(CPS * s + 1)
            conv_range(s, HMW, HMW)
            down(CPS * s + 2)
            down(CPS * s + 3)
        # last sequence at chunk granularity to shorten the drain tail:
        # conv of chunk c needs the first cols of chunk c+1's copies
        s = NSEQ - 1
        produce(CPS * s + 0)
        produce(CPS * s + 1)
        conv_range(s, 0, T)
        produce(CPS * s + 2)
        conv_range(s, T, T)
        down(CPS * s + 0)
        produce(CPS * s + 3)
        conv_range(s, 2 * T, T)
        down(CPS * s + 1)
        conv_range(s, 3 * T, T)
        down(CPS * s + 2)
        down(CPS * s + 3)

    nc.compile()
    return nc


def _prep_inputs(x, Wg, bgv, Wu, buv, convg_w, convg_b, convu_w, convu_b, Wd):
    """Host-side shard/layout. Returns list of per-core in_maps."""
    bf16 = ml_dtypes.bfloat16
    x = np.ascontiguousarray(x, np.float32)
    # [B, L, D] -> [B, KSUB, 128, L] -> chunks [NCH, 128, KSUB, T]
    xt = x.transpose(0, 2, 1).reshape(B, KSUB, 128, L)
    xTc = np.stack([
        xt[i // CPS, :, :, (i % CPS) * T:(i % CPS + 1) * T].transpose(1, 0, 2)
        for i in range(NCH)
    ]).astype(bf16)

    def colsplit(w, c):      # [D, F] -> per-core [128, KSUB, FS]
        s = np.asarray(w, np.float32)[:, c * FS:(c + 1) * FS]
        return np.ascontiguousarray(
            s.reshape(KSUB, 128, FS).transpose(1, 0, 2)).astype(bf16)

    def vecsplit(v, c):      # [F] -> [128, GRP] f32
        return np.ascontiguousarray(
            np.asarray(v, np.float32)[c * FS:(c + 1) * FS].reshape(GRP, 128).T)

    def tapsplit(w, c):      # [F, 1, K] -> [128, GRP, K] f32
        return np.ascontiguousarray(
            np.asarray(w, np.float32)[c * FS:(c + 1) * FS, 0, :]
            .reshape(GRP, 128, K).transpose(1, 0, 2))

    in_maps = []
    for c in range(NCORES):
        wdS = np.asarray(Wd, np.float32)[c * FS:(c + 1) * FS, :]
        in_maps.append({
            "xTc": xTc,
            "wgS": colsplit(Wg, c),
            "wuS": colsplit(Wu, c),
            "wdS": np.ascontiguousarray(
                wdS.reshape(GRP, 128, D).transpose(1, 0, 2)).astype(bf16),
            "bgS": vecsplit(bgv, c),
            "buS": vecsplit(buv, c),
            "cgwS": tapsplit(convg_w, c),
            "cuwS": tapsplit(convu_w, c),
            "cgbS": vecsplit(convg_b, c),
            "cubS": vecsplit(convu_b, c),
        })
    return in_maps


def run_on_cores(in_maps, **kwargs):
    if "nc" not in _cache:
        _cache["nc"] = _build_program()
    return run_bass_kernel_spmd(_cache["nc"], in_maps,
                                core_ids=list(range(NCORES)), **kwargs)


def kernel(x, Wg, bg, Wu, bu, convg_w, convg_b, convu_w, convu_b, Wd, bd):
    in_maps = _prep_inputs(x, Wg, bg, Wu, bu, convg_w, convg_b,
                           convu_w, convu_b, Wd)
    res = run_on_cores(in_maps)
    acc = np.zeros((D, B * L), np.float32)
    for r in res.results:
        acc += np.asarray(r["yT"], np.float32)
    acc += np.asarray(bd, np.float32)[:, None]
    return np.ascontiguousarray(acc.T.reshape(B, L, D)).astype(np.float32)
